# revision 22
# baseline (speedup 1.0000x reference)
"""TRN2 Bass kernel for nn_COV_75359496176097.

reference():
    B2 = B[0]                               # (8192, 8192)
    rn = sqrt(1 / sum(B2*B2, axis=1))       # row norms
    A  = rn * B2 * exp(tile(logstd, 64))[:, None]
    samples = tile(mu,64) + einsum('mk,bk->bm', A, eps[:,:,0])
    returns (mu_out, logvar, samples), each (128, 64, 128)

Strategy: shard B by rows across 8 cores (1024 rows each, no collectives).
Each core computes out[b, r] = sum_k eps[k, b] * B[r, k] on the PE
(eps k-tile stationary fp32r, B^T k-tile moving fp32r, PSUM-accumulated
over 64 k-tiles). Row norms: ACT squares each B^T tile (fp32), DVE
accumulates the squares elementwise across k-tiles (ping-pong pair of
accumulators to avoid back-to-back pipeline hazards), and a single pair
of all-ones fp32 matmuls in the tail does the 128-partition reduction —
broadcast across all output partitions for free. Epilogue applies
out = acc * sqrt(1/sumsq) * exp(logstd_rep) + mu_rep and DMAs out.

Raw Bass (not Tile): hardware allows at most ONE semaphore wait per
instruction, and this dataflow (each DMA'd tile consumed by PE and ACT)
needs transitive cross-engine reasoning Tile doesn't do. Manual scheme:
per-slot DMA-completion semaphores; DVE's accumulate for tile t waits on
both PE (via a nop) and ACT, so "DVE retired tile t" implies every
consumer of slot t is done; the DMA issuer throttles on that single DVE
semaphore.

Each k-tile's B^T slice and eps^T slice are packed side by side in one
host-prepared tensor so a k-tile needs exactly one DMA.
"""

import sys
from contextlib import ExitStack

if "/opt/trn_rl_repo" not in sys.path:
    sys.path.insert(0, "/opt/trn_rl_repo")

import numpy as np

import concourse.bacc as bacc
import concourse.mybir as mybir
from concourse import bass_utils
from concourse.dve_ops import RECIPROCAL_APPROX_NR

Z = 128
NS = 64
M = Z * NS          # 8192
BATCH = 128
NCORES = 8
RPC = M // NCORES   # 1024 rows of B per core
KT = M // 128       # 64 k-tiles
W = RPC + BATCH     # 1152 packed row width
NB = 12             # B-tile SBUF slots (DMA prefetch depth)
SPLITS = {0: 4, 1: 2, 2: 2}  # first tiles DMA'd in chunks (parallel ramp-up)
EXTRA = {s: 16 * (n - 1) for s, n in SPLITS.items()}

f32 = mybir.dt.float32
f32r = mybir.dt.float32r
bf16 = mybir.dt.bfloat16

_nc_cache = {}


def _dma_need(t):
    """semaphore threshold for tile t's slot DMA(s) to have completed"""
    return 16 * (t // NB + 1) + EXTRA.get(t % NB, 0)


def _build():
    nc = bacc.Bacc("TRN2", debug=False)

    bte_d = nc.dram_tensor("bte", (M, W), f32r, kind="ExternalInput")
    els_d = nc.dram_tensor("els", (BATCH, RPC), f32, kind="ExternalInput")
    mu_d = nc.dram_tensor("mu", (BATCH, RPC), f32, kind="ExternalInput")
    out_d = nc.dram_tensor("out", (BATCH, RPC), f32, kind="ExternalOutput")

    with ExitStack() as ctx:
        e = ctx.enter_context
        slots = [e(nc.sbuf_tensor(f"slot{i}", [128, W], f32r)) for i in range(NB)]
        sq = [e(nc.sbuf_tensor(f"sq{i}", [128, RPC], bf16)) for i in range(NB)]
        ones = e(nc.sbuf_tensor("ones", [128, 128], bf16))
        els_sb = e(nc.sbuf_tensor("els_sb", [128, RPC], f32))
        mu_sb = e(nc.sbuf_tensor("mu_sb", [128, RPC], f32))
        inv_sb = e(nc.sbuf_tensor("inv_sb", [128, RPC], f32))
        rn_sb = e(nc.sbuf_tensor("rn_sb", [128, RPC], f32))
        scale_sb = e(nc.sbuf_tensor("scale_sb", [128, RPC], f32))
        out_sb = e(nc.sbuf_tensor("out_sb", [128, RPC], f32))
        acc = e(nc.psum_tensor([128, RPC], f32))
        nrm = e(nc.psum_tensor([128, RPC], f32))
        warm_ps = e(nc.psum_tensor([128, 128], f32))

        s_dma = [e(nc.semaphore(name=f"s_dma{i}")) for i in range(NB)]
        s_cst = e(nc.semaphore(name="s_cst"))
        s_pe = e(nc.semaphore(name="s_pe"))
        s_dve = e(nc.semaphore(name="s_dve"))
        s_act = e(nc.semaphore(name="s_act"))
        s_x = e(nc.semaphore(name="s_x"))
        s_acc = e(nc.semaphore(name="s_acc"))
        s_wm = e(nc.semaphore(name="s_wm"))
        s_out = e(nc.semaphore(name="s_out"))
        s_od = e(nc.semaphore(name="s_od"))

        block = e(nc.Block())

        @block.sync
        def _(sync):
            for t in range(KT):
                sl = slice(t * 128, (t + 1) * 128)
                if t == NB:
                    # constants only needed by the epilogue; issue after the
                    # first wave of B-tile DMAs so the PE starts sooner
                    sync.dma_start(els_sb[:], els_d.ap()[:, :]).then_inc(
                        s_cst, 16
                    )
                    sync.dma_start(mu_sb[:], mu_d.ap()[:, :]).then_inc(
                        s_cst, 16
                    )
                if t >= NB:
                    # slot free once PE's norm matmul of tile t-NB retired
                    # (transitively implies DVE's square is done too)
                    sync.wait_ge(s_pe, t - NB + 1)
                nchunk = SPLITS.get(t, 1)
                p = 128 // nchunk
                for ci in range(nchunk):
                    sync.dma_start(
                        slots[t % NB][ci * p:(ci + 1) * p, :],
                        bte_d.ap()[sl, :][ci * p:(ci + 1) * p, :],
                    ).then_inc(s_dma[t % NB], 16)
                if 3 <= t < NB:
                    # pace the ramp-up: issuing the whole first wave at full
                    # queue rate bloats the queue (tiles 12+ then arrive late)
                    # and slams the HBM stack shared with the paired core
                    for _ in range(24):
                        sync.nop()
            sync.wait_ge(s_out, 1)
            for ci in range(4):
                sync.dma_start(
                    out_d.ap()[ci * 32:(ci + 1) * 32, :],
                    out_sb[ci * 32:(ci + 1) * 32, :],
                ).then_inc(s_od, 16)
            sync.wait_ge(s_od, 64)
            sync.nop()

        @block.tensor
        def _(tensor):
            # warmup matmuls: pin the PE HAM activity monitor to the warm
            # (full-clock) state before the first B tile lands
            tensor.wait_ge(s_wm, 1)
            for _ in range(24):
                nc.tensor.matmul(
                    warm_ps[:, 0:128], ones[:], ones[:], start=True, stop=True
                )
            for t in range(KT):
                st, sp = t == 0, t == KT - 1
                s = t % NB
                tensor.wait_ge(s_dma[s], _dma_need(t))
                eps_v = slots[s][:, RPC:W]
                for h in range(RPC // 512):
                    hs = slice(h * 512, (h + 1) * 512)
                    ins = nc.tensor.matmul(
                        acc[:, hs], eps_v, slots[s][:, hs], start=st, stop=sp
                    )
                if sp:
                    # lets DVE start acc*els while the norm matmuls finish
                    ins.then_inc(s_acc, 1)
                # bf16 norm matmuls double as LDW shadow for the fp32r pair
                tensor.wait_ge(s_dve, t + 1)
                for h in range(RPC // 512):
                    hs = slice(h * 512, (h + 1) * 512)
                    ins = nc.tensor.matmul(
                        nrm[:, hs], ones[:], sq[s][:, hs], start=st, stop=sp
                    )
                ins.then_inc(s_pe, 1)

        @block.scalar
        def _(scalar):
            scalar.wait_ge(s_pe, KT)
            nc.scalar.sqrt(inv_sb[:], nrm[:]).then_inc(s_x, 1)

        @block.vector
        def _(vector):
            nc.vector.memset(ones[:], 1.0).then_inc(s_wm, 1)
            for t in range(KT):
                s = t % NB
                # the slot DMA only fired after PE retired tile t-NB, so the
                # sq[s] anti-dependency (PE read of square t-NB) is implied
                vector.wait_ge(s_dma[s], _dma_need(t))
                btf = slots[s][:, 0:RPC].bitcast(f32)
                nc.vector.tensor_mul(sq[s][:], btf, btf).then_inc(s_dve, 1)
            # epilogue: out = (acc*els) / sqrt(nrm) + mu.  acc*els overlaps
            # the final norm matmuls and the ACT sqrt.
            vector.wait_ge(s_cst, 32)
            vector.drain()
            vector.wait_ge(s_acc, 1)
            nc.vector.tensor_mul(scale_sb[:], acc[:], els_sb[:])
            vector.drain()
            # rn = 1/sqrt(nrm) at ~2 ULP: ACT sqrt, then 2-op approx recip
            vector.wait_ge(s_x, 1)
            nc.vector.reciprocal_approx_fast(out=rn_sb[:], in_=inv_sb[:])
            vector.drain()
            nc.vector._custom_dve(
                RECIPROCAL_APPROX_NR,
                out=out_sb[:],
                in0=inv_sb[:],
                in1=rn_sb[:],
                s0=2.0,
            )
            vector.drain()
            nc.vector.tensor_mul(out_sb[:], scale_sb[:], out_sb[:])
            vector.drain()
            nc.vector.tensor_add(out_sb[:], out_sb[:], mu_sb[:]).then_inc(
                s_out, 1
            )

    nc.compile()
    return nc


def _get_nc():
    if "nc" not in _nc_cache:
        _nc_cache["nc"] = _build()
    return _nc_cache["nc"]


def _prep_inputs(mu, logstd, B, eps):
    B2 = B[0]
    epst = np.ascontiguousarray(eps[:, :, 0].T)        # (M, BATCH)
    mu_rep = np.tile(mu[0], NS)                        # (M,)
    logstd_rep = np.tile(logstd, NS)                   # (M,)
    els_rep = np.exp(logstd_rep).astype(np.float32)    # (M,)

    in_maps = []
    for c in range(NCORES):
        rows = slice(c * RPC, (c + 1) * RPC)
        bte = np.empty((M, W), dtype=np.float32)
        bte[:, 0:RPC] = B2[rows, :].T
        bte[:, RPC:W] = epst
        in_maps.append(
            {
                "bte": bte,
                "els": np.ascontiguousarray(
                    np.broadcast_to(els_rep[rows][None, :], (BATCH, RPC))
                ),
                "mu": np.ascontiguousarray(
                    np.broadcast_to(mu_rep[rows][None, :], (BATCH, RPC))
                ),
            }
        )
    return in_maps, mu_rep, logstd_rep


def _run(mu, logstd, B, eps, batch_size, trace=False, trace_kwargs=None):
    mu = np.asarray(mu, dtype=np.float32)
    logstd = np.asarray(logstd, dtype=np.float32)
    B = np.asarray(B, dtype=np.float32)
    eps = np.asarray(eps, dtype=np.float32)
    b = int(batch_size)
    assert B.shape == (1, M, M) and eps.shape == (b, M, 1) and b == BATCH

    in_maps, mu_rep, logstd_rep = _prep_inputs(mu, logstd, B, eps)

    nc = _get_nc()
    kw = {}
    if trace:
        kw = dict(trace=True, trace_cores=list(range(NCORES)))
        if trace_kwargs:
            kw.update(trace_kwargs)
    res = bass_utils.run_bass_kernel_spmd(
        nc, in_maps, core_ids=list(range(NCORES)), **kw
    )

    samples_bm = np.concatenate(
        [res.results[c]["out"] for c in range(NCORES)], axis=1
    )  # (BATCH, M)
    samples = samples_bm.reshape(b, NS, Z)
    mu_out = np.broadcast_to(mu_rep[None, :], (b, M)).reshape(b, NS, Z).copy()
    logvar = (
        np.broadcast_to(2.0 * logstd_rep[None, :], (b, M)).reshape(b, NS, Z).copy()
    )
    return (mu_out, logvar, samples), res


def kernel(mu, logstd, B, eps, batch_size):
    outs, _ = _run(mu, logstd, B, eps, batch_size, trace=False)
    return outs


# revision 24
# speedup vs baseline: 1.0373x; 1.0373x over previous
"""TRN2 Bass kernel for nn_COV_75359496176097.

reference():
    B2 = B[0]                               # (8192, 8192)
    rn = sqrt(1 / sum(B2*B2, axis=1))       # row norms
    A  = rn * B2 * exp(tile(logstd, 64))[:, None]
    samples = tile(mu,64) + einsum('mk,bk->bm', A, eps[:,:,0])
    returns (mu_out, logvar, samples), each (128, 64, 128)

Strategy: shard B by rows across 8 cores (1024 rows each, no collectives).
Each core computes out[b, r] = sum_k eps[k, b] * B[r, k] on the PE
(eps k-tile stationary fp32r, B^T k-tile moving fp32r, PSUM-accumulated
over 64 k-tiles; fp32r streams at full fp32-ish precision, measured
~8e-5 max rel err). Row norms ride along: DVE squares each B^T tile to
bf16 and an all-ones bf16 stationary matmul accumulates the column sums
into a second PSUM bank — replicating them across all 128 output
partitions for free, and doubling as pipeline shadow for the fp32 weight
loads of the fp32r matmuls. A 24-matmul warmup keeps the PE's HAM clock
monitor in the full-speed state before the first B tile lands. Epilogue:
out = (acc*els) * 1/sqrt(nrm) + mu with acc*els overlapped into the loop
tail, ACT sqrt, and a two-op ~2ULP approximate reciprocal on DVE.

Raw Bass (not Tile): hardware allows at most ONE semaphore wait per
instruction, and this dataflow (each DMA'd tile consumed by PE and DVE)
needs transitive cross-engine reasoning Tile doesn't do. Manual scheme:
per-slot DMA-completion semaphores; PE's norm matmul for tile t waits on
DVE's square, so "PE retired tile t" implies every consumer of slot t is
done; the DMA issuer throttles on that single PE semaphore.

Each k-tile's B^T slice and eps^T slice are packed side by side in one
host-prepared tensor so a k-tile needs exactly one DMA.
"""

import sys
from contextlib import ExitStack

if "/opt/trn_rl_repo" not in sys.path:
    sys.path.insert(0, "/opt/trn_rl_repo")

import numpy as np

import concourse.bacc as bacc
import concourse.mybir as mybir
from concourse import bass_utils
from concourse.dve_ops import RECIPROCAL_APPROX_NR

Z = 128
NS = 64
M = Z * NS          # 8192
BATCH = 128
NCORES = 8
RPC = M // NCORES   # 1024 rows of B per core
KT = M // 128       # 64 k-tiles
W = RPC + BATCH     # 1152 packed row width
NB = 12             # B-tile SBUF slots (DMA prefetch depth)
SPLITS = {0: 4, 1: 2, 2: 2}  # first tiles DMA'd in chunks (parallel ramp-up)
EXTRA = {s: 16 * (n - 1) for s, n in SPLITS.items()}

f32 = mybir.dt.float32
f32r = mybir.dt.float32r
bf16 = mybir.dt.bfloat16

_nc_cache = {}


def _dma_need(t):
    """semaphore threshold for tile t's slot DMA(s) to have completed"""
    return 16 * (t // NB + 1) + EXTRA.get(t % NB, 0)


def _build():
    nc = bacc.Bacc("TRN2", debug=False)

    bte_d = nc.dram_tensor("bte", (M, W), f32r, kind="ExternalInput")
    els_d = nc.dram_tensor("els", (BATCH, RPC), f32, kind="ExternalInput")
    mu_d = nc.dram_tensor("mu", (BATCH, RPC), f32, kind="ExternalInput")
    out_d = nc.dram_tensor("out", (BATCH, RPC), f32, kind="ExternalOutput")

    with ExitStack() as ctx:
        e = ctx.enter_context
        slots = [e(nc.sbuf_tensor(f"slot{i}", [128, W], f32r)) for i in range(NB)]
        sq = [e(nc.sbuf_tensor(f"sq{i}", [128, RPC], bf16)) for i in range(NB)]
        ones = e(nc.sbuf_tensor("ones", [128, 128], bf16))
        els_sb = e(nc.sbuf_tensor("els_sb", [128, RPC], f32))
        mu_sb = e(nc.sbuf_tensor("mu_sb", [128, RPC], f32))
        inv_sb = e(nc.sbuf_tensor("inv_sb", [128, RPC], f32))
        rn_sb = e(nc.sbuf_tensor("rn_sb", [128, RPC], f32))
        scale_sb = e(nc.sbuf_tensor("scale_sb", [128, RPC], f32))
        out_sb = e(nc.sbuf_tensor("out_sb", [128, RPC], f32))
        acc = e(nc.psum_tensor([128, RPC], f32))
        nrm = e(nc.psum_tensor([128, RPC], f32))
        warm_ps = e(nc.psum_tensor([128, 128], f32))

        s_dma = [e(nc.semaphore(name=f"s_dma{i}")) for i in range(NB)]
        s_cst = e(nc.semaphore(name="s_cst"))
        s_pe = e(nc.semaphore(name="s_pe"))
        s_dve = e(nc.semaphore(name="s_dve"))
        s_act = e(nc.semaphore(name="s_act"))
        s_x = e(nc.semaphore(name="s_x"))
        s_acc = e(nc.semaphore(name="s_acc"))
        s_wm = e(nc.semaphore(name="s_wm"))
        s_out = e(nc.semaphore(name="s_out"))
        s_od = e(nc.semaphore(name="s_od"))

        block = e(nc.Block())

        @block.sync
        def _(sync):
            for t in range(KT):
                sl = slice(t * 128, (t + 1) * 128)
                if t == NB:
                    # constants only needed by the epilogue; issue after the
                    # first wave of B-tile DMAs so the PE starts sooner
                    sync.dma_start(els_sb[:], els_d.ap()[:, :]).then_inc(
                        s_cst, 16
                    )
                    sync.dma_start(mu_sb[:], mu_d.ap()[:, :]).then_inc(
                        s_cst, 16
                    )
                if t >= NB:
                    # slot free once PE's norm matmul of tile t-NB retired
                    # (transitively implies DVE's square is done too)
                    sync.wait_ge(s_pe, t - NB + 1)
                nchunk = SPLITS.get(t, 1)
                p = 128 // nchunk
                for ci in range(nchunk):
                    sync.dma_start(
                        slots[t % NB][ci * p:(ci + 1) * p, :],
                        bte_d.ap()[sl, :][ci * p:(ci + 1) * p, :],
                    ).then_inc(s_dma[t % NB], 16)
            sync.wait_ge(s_out, 1)
            for ci in range(4):
                sync.dma_start(
                    out_d.ap()[ci * 32:(ci + 1) * 32, :],
                    out_sb[ci * 32:(ci + 1) * 32, :],
                ).then_inc(s_od, 16)
            sync.wait_ge(s_od, 64)
            sync.nop()

        @block.tensor
        def _(tensor):
            # warmup matmuls: pin the PE HAM activity monitor to the warm
            # (full-clock) state before the first B tile lands
            tensor.wait_ge(s_wm, 1)
            for _ in range(24):
                nc.tensor.matmul(
                    warm_ps[:, 0:128], ones[:], ones[:], start=True, stop=True
                )
            for t in range(KT):
                st, sp = t == 0, t == KT - 1
                s = t % NB
                tensor.wait_ge(s_dma[s], _dma_need(t))
                eps_v = slots[s][:, RPC:W]
                for h in range(RPC // 512):
                    hs = slice(h * 512, (h + 1) * 512)
                    ins = nc.tensor.matmul(
                        acc[:, hs], eps_v, slots[s][:, hs], start=st, stop=sp
                    )
                if sp:
                    # lets DVE start acc*els while the norm matmuls finish
                    ins.then_inc(s_acc, 1)
                # bf16 norm matmuls double as LDW shadow for the fp32r pair
                tensor.wait_ge(s_dve, t + 1)
                for h in range(RPC // 512):
                    hs = slice(h * 512, (h + 1) * 512)
                    ins = nc.tensor.matmul(
                        nrm[:, hs], ones[:], sq[s][:, hs], start=st, stop=sp
                    )
                ins.then_inc(s_pe, 1)

        @block.scalar
        def _(scalar):
            scalar.wait_ge(s_pe, KT)
            nc.scalar.sqrt(inv_sb[:], nrm[:]).then_inc(s_x, 1)

        @block.vector
        def _(vector):
            nc.vector.memset(ones[:], 1.0).then_inc(s_wm, 1)
            for t in range(KT):
                s = t % NB
                # the slot DMA only fired after PE retired tile t-NB, so the
                # sq[s] anti-dependency (PE read of square t-NB) is implied
                vector.wait_ge(s_dma[s], _dma_need(t))
                btf = slots[s][:, 0:RPC].bitcast(f32)
                nc.vector.tensor_mul(sq[s][:], btf, btf).then_inc(s_dve, 1)
            # epilogue: out = (acc*els) / sqrt(nrm) + mu.  acc*els overlaps
            # the final norm matmuls and the ACT sqrt.
            vector.wait_ge(s_cst, 32)
            vector.drain()
            vector.wait_ge(s_acc, 1)
            nc.vector.tensor_mul(scale_sb[:], acc[:], els_sb[:])
            vector.drain()
            # rn = 1/sqrt(nrm) at ~2 ULP: ACT sqrt, then 2-op approx recip
            vector.wait_ge(s_x, 1)
            nc.vector.reciprocal_approx_fast(out=rn_sb[:], in_=inv_sb[:])
            vector.drain()
            nc.vector._custom_dve(
                RECIPROCAL_APPROX_NR,
                out=out_sb[:],
                in0=inv_sb[:],
                in1=rn_sb[:],
                s0=2.0,
            )
            vector.drain()
            nc.vector.tensor_mul(out_sb[:], scale_sb[:], out_sb[:])
            vector.drain()
            nc.vector.tensor_add(out_sb[:], out_sb[:], mu_sb[:]).then_inc(
                s_out, 1
            )

    nc.compile()
    return nc


def _get_nc():
    if "nc" not in _nc_cache:
        _nc_cache["nc"] = _build()
    return _nc_cache["nc"]


def _prep_inputs(mu, logstd, B, eps):
    B2 = B[0]
    epst = np.ascontiguousarray(eps[:, :, 0].T)        # (M, BATCH)
    mu_rep = np.tile(mu[0], NS)                        # (M,)
    logstd_rep = np.tile(logstd, NS)                   # (M,)
    els_rep = np.exp(logstd_rep).astype(np.float32)    # (M,)

    in_maps = []
    for c in range(NCORES):
        rows = slice(c * RPC, (c + 1) * RPC)
        bte = np.empty((M, W), dtype=np.float32)
        bte[:, 0:RPC] = B2[rows, :].T
        bte[:, RPC:W] = epst
        in_maps.append(
            {
                "bte": bte,
                "els": np.ascontiguousarray(
                    np.broadcast_to(els_rep[rows][None, :], (BATCH, RPC))
                ),
                "mu": np.ascontiguousarray(
                    np.broadcast_to(mu_rep[rows][None, :], (BATCH, RPC))
                ),
            }
        )
    return in_maps, mu_rep, logstd_rep


def _run(mu, logstd, B, eps, batch_size, trace=False, trace_kwargs=None):
    mu = np.asarray(mu, dtype=np.float32)
    logstd = np.asarray(logstd, dtype=np.float32)
    B = np.asarray(B, dtype=np.float32)
    eps = np.asarray(eps, dtype=np.float32)
    b = int(batch_size)
    assert B.shape == (1, M, M) and eps.shape == (b, M, 1) and b == BATCH

    in_maps, mu_rep, logstd_rep = _prep_inputs(mu, logstd, B, eps)

    nc = _get_nc()
    kw = {}
    if trace:
        kw = dict(trace=True, trace_cores=list(range(NCORES)))
        if trace_kwargs:
            kw.update(trace_kwargs)
    res = bass_utils.run_bass_kernel_spmd(
        nc, in_maps, core_ids=list(range(NCORES)), **kw
    )

    samples_bm = np.concatenate(
        [res.results[c]["out"] for c in range(NCORES)], axis=1
    )  # (BATCH, M)
    samples = samples_bm.reshape(b, NS, Z)
    mu_out = np.broadcast_to(mu_rep[None, :], (b, M)).reshape(b, NS, Z).copy()
    logvar = (
        np.broadcast_to(2.0 * logstd_rep[None, :], (b, M)).reshape(b, NS, Z).copy()
    )
    return (mu_out, logvar, samples), res


def kernel(mu, logstd, B, eps, batch_size):
    outs, _ = _run(mu, logstd, B, eps, batch_size, trace=False)
    return outs
